# revision 10
# baseline (speedup 1.0000x reference)
"""Trainium2 Bass kernel for BaseWindowAttention.

Problem (hardcoded): x [2,8,64,64,256] f32, w_qkv [256,768], w_out [256,256],
b_out [256], pos_embedding [15,15], window_size 8, heads 8, dim_head 32.

Strategy (v2):
- Data parallel: 16 (b,l) images over 8 cores -> 2 images/core.
- Host: window-major channel-first bf16 transpose of x; fold softmax scale
  into w_q; precompute exp(bias) 2-window super-tile (off-diagonal zeros kill
  cross-window attention terms).
- Device per core, per strip of 512 tokens (8 windows):
  * q/k projection -> [o,t] layout; v projection -> [t,h,c] layout.
  * window-pair dots as row-group-packed [32,x]x[32,x] matmuls (keys on
    partitions), ACT exp, Pool multiply by exp(bias) mask tile.
  * AV computed TRANSPOSED: avT[hc,t] = v.T @ edm via v-stationary matmuls
    col-packed 4 heads per [128,4,128] PSUM tile (tile_position=(0,32h)).
    No PE transpose needed before the out-projection.
  * softmax denominators via all-ones [128,32] stationary matmuls into a
    second [128,4,128] PSUM tile (each head's den replicated across its 32
    rows -> partition-aligned normalize).
  * normalize: DVE reciprocal + DVE multiply -> attn[hc,t] bf16 in SBUF.
  * out-projection with b_out folded in as a K=1 rank-1 matmul; one batched
    output DMA per strip.
"""

import os
import sys
import numpy as np

sys.path.insert(0, "/opt/trn_rl_repo")
os.environ.setdefault("JAX_PLATFORMS", "")

import ml_dtypes

BF16 = ml_dtypes.bfloat16

B, L, H, W, C = 2, 8, 64, 64, 256
WS = 8
NHEADS = 8
CH = 32
N_CORES = 8
IMG = B * L                 # 16 images
IMG_PER_CORE = IMG // N_CORES
T_IMG = H * W               # 4096 tokens per image
STRIP = 512                 # tokens per strip (8 windows)
N_STRIPS = T_IMG // STRIP   # 8
NWP = STRIP // 128          # 4 window pairs per strip

_CACHE = {}


def _relative_indices(ws):
    idx = np.array([[i, j] for i in range(ws) for j in range(ws)])
    rel = idx[None, :, :] - idx[:, None, :] + ws - 1
    return rel


def _build_kernel(repeat=1):
    import concourse.bass as bass
    import concourse.mybir as mybir
    import concourse.tile as tile
    from concourse import bacc

    dt = mybir.dt
    nc = bacc.Bacc("TRN2", target_bir_lowering=False, debug=False)

    xT = nc.dram_tensor("xT", [IMG_PER_CORE, C, T_IMG], dt.bfloat16,
                        kind="ExternalInput").ap()
    wqk = nc.dram_tensor("wqk", [C, 512], dt.bfloat16, kind="ExternalInput").ap()
    wv = nc.dram_tensor("wv", [C, C], dt.bfloat16, kind="ExternalInput").ap()
    wout = nc.dram_tensor("wout", [C, C], dt.bfloat16, kind="ExternalInput").ap()
    bout = nc.dram_tensor("bout", [1, C], dt.bfloat16, kind="ExternalInput").ap()
    ebrep = nc.dram_tensor("ebrep", [128, 1024], dt.bfloat16,
                           kind="ExternalInput").ap()
    ones32 = nc.dram_tensor("ones32", [128, 32], dt.bfloat16,
                            kind="ExternalInput").ap()
    ones1 = nc.dram_tensor("ones1", [1, 128], dt.bfloat16,
                           kind="ExternalInput").ap()
    out = nc.dram_tensor("out", [IMG_PER_CORE, T_IMG, C], dt.bfloat16,
                         kind="ExternalOutput").ap()

    EXP = mybir.ActivationFunctionType.Exp

    def env(k, d):
        return int(os.environ.get(k, str(d)))

    with tile.TileContext(nc) as tc:
        from contextlib import ExitStack
        with ExitStack() as ctx:
            consts = ctx.enter_context(tc.tile_pool(name="consts", bufs=1))
            xp = ctx.enter_context(tc.tile_pool(name="xp", bufs=env("XP_BUFS", 3)))
            qkp = ctx.enter_context(tc.tile_pool(name="qkp", bufs=env("QKP_BUFS", 8)))
            vp = ctx.enter_context(tc.tile_pool(name="vp", bufs=env("VP_BUFS", 6)))
            edp = ctx.enter_context(tc.tile_pool(name="edp", bufs=env("EDP_BUFS", 4)))
            emp = ctx.enter_context(tc.tile_pool(name="emp", bufs=env("EMP_BUFS", 4)))
            rcp = ctx.enter_context(tc.tile_pool(name="rcp", bufs=env("RCP_BUFS", 3)))
            atp = ctx.enter_context(tc.tile_pool(name="atp", bufs=env("ATP_BUFS", 4)))
            fop = ctx.enter_context(tc.tile_pool(name="fop", bufs=env("FOP_BUFS", 3)))
            # PSUM: pvA {qk, v} 2 banks + pvB {dots} 4 banks + pvC {avT, den, ops} 2
            pvA = ctx.enter_context(tc.tile_pool(
                name="pvA", bufs=env("PSA_BUFS", 2), space="PSUM"))
            pvB = ctx.enter_context(tc.tile_pool(
                name="pvB", bufs=env("PSB_BUFS", 2), space="PSUM"))
            pvC = ctx.enter_context(tc.tile_pool(
                name="pvC", bufs=env("PSC_BUFS", 2), space="PSUM"))

            # ---- constants into SBUF
            wqk_sb, wv_sb, wout_sb = [], [], []
            for kk in range(2):
                wqk_t = consts.tile([128, 512], dt.bfloat16, tag=f"wqk{kk}")
                nc.sync.dma_start(out=wqk_t, in_=wqk[kk * 128:(kk + 1) * 128, :])
                wqk_sb.append(wqk_t)
                wv_t = consts.tile([128, 256], dt.bfloat16, tag=f"wv{kk}")
                nc.sync.dma_start(out=wv_t, in_=wv[kk * 128:(kk + 1) * 128, :])
                wv_sb.append(wv_t)
                wout_t = consts.tile([128, 256], dt.bfloat16, tag=f"wout{kk}")
                nc.sync.dma_start(out=wout_t, in_=wout[kk * 128:(kk + 1) * 128, :])
                wout_sb.append(wout_t)
            eb_sb = consts.tile([128, 2, 512], dt.bfloat16, tag="eb")
            nc.sync.dma_start(out=eb_sb, in_=ebrep.rearrange("p (r c) -> p r c", r=2))
            o32_sb = consts.tile([128, 32], dt.bfloat16, tag="o32")
            nc.sync.dma_start(out=o32_sb, in_=ones32)
            ones_sb = consts.tile([1, 128], dt.bfloat16, tag="ones")
            nc.sync.dma_start(out=ones_sb, in_=ones1)
            bout_sb = consts.tile([1, 256], dt.bfloat16, tag="bout")
            nc.sync.dma_start(out=bout_sb, in_=bout)

            for img_rep in range(IMG_PER_CORE * repeat):
                img = img_rep % IMG_PER_CORE
                for s in range(N_STRIPS):
                    t0 = s * STRIP
                    xa = xp.tile([128, STRIP], dt.bfloat16, tag="xa")
                    nc.sync.dma_start(out=xa, in_=xT[img, 0:128, t0:t0 + STRIP])
                    xb = xp.tile([128, STRIP], dt.bfloat16, tag="xb")
                    nc.sync.dma_start(out=xb, in_=xT[img, 128:256, t0:t0 + STRIP])

                    # ---- q/k projection: out [o=128 (4 heads), t=512]
                    # bf16 PSUM: halves DVE copy cost (2x mode on 16-bit src).
                    qk_sb = []
                    for ot in range(4):  # q(h0-3), q(h4-7), k(h0-3), k(h4-7)
                        qkps = pvA.tile([128, STRIP], dt.float32, tag="psA")
                        nc.tensor.matmul(qkps, wqk_sb[0][:, ot * 128:(ot + 1) * 128],
                                         xa, start=True, stop=False)
                        nc.tensor.matmul(qkps, wqk_sb[1][:, ot * 128:(ot + 1) * 128],
                                         xb, start=False, stop=True)
                        qk_t = qkp.tile([128, STRIP], dt.bfloat16, tag="qk_t")
                        nc.vector.tensor_copy(qk_t, qkps)
                        qk_sb.append(qk_t)

                    # ---- v projection: out [t=128, 8, 32]
                    v_sb = []
                    for tb in range(NWP):
                        vps = pvA.tile([128, NHEADS, CH], dt.float32, tag="psA")
                        nc.tensor.matmul(vps, xa[:, tb * 128:(tb + 1) * 128],
                                         wv_sb[0], start=True, stop=False)
                        nc.tensor.matmul(vps, xb[:, tb * 128:(tb + 1) * 128],
                                         wv_sb[1], start=False, stop=True)
                        v3 = vp.tile([128, NHEADS, CH], dt.bfloat16, tag="v3")
                        nc.vector.tensor_copy(v3, vps)
                        v_sb.append(v3)

                    # ---- dots -> exp -> mask-mult, per (hg, half)
                    # edm[j, i] layout: keys j on partitions.
                    edm_sb = {}
                    for hg in range(2):
                        for half in range(2):
                            dps = pvB.tile([128, 2, 512], dt.float32, tag="dps")
                            for wp in range(NWP):
                                c0 = wp * 128
                                for r2 in range(2):
                                    rg = 2 * half + r2
                                    nc.tensor.matmul(
                                        dps[:, r2, c0:c0 + 128],
                                        qk_sb[2 + hg][32 * rg:32 * rg + 32,
                                                      c0:c0 + 128],
                                        qk_sb[hg][32 * rg:32 * rg + 32,
                                                  c0:c0 + 128],
                                        start=True, stop=True,
                                        tile_position=(32 * rg, 0),
                                    )
                            ed = edp.tile([128, 2, 512], dt.bfloat16, tag="ed")
                            nc.scalar.activation(ed, dps, EXP)
                            edm = emp.tile([128, 2, 512], dt.bfloat16, tag="edm")
                            nc.gpsimd.tensor_mul(edm, ed,
                                                 eb_sb)
                            edm_sb[(hg, half)] = edm

                    # ---- avT + den: [hc, t] via v-stationary col-packed MMs
                    attn_sb = []
                    for hg in range(2):
                        avps = pvC.tile([128, NWP, 128], dt.float32, tag="psC")
                        dnps = pvC.tile([128, NWP, 128], dt.float32, tag="psC")
                        for hh in range(4):
                            # one N=512 den matmul covers all 4 window pairs
                            nc.tensor.matmul(
                                dnps[32 * hh:32 * hh + 32, :, :],
                                o32_sb,
                                edm_sb[(hg, hh // 2)][:, hh % 2, :],
                                start=True, stop=True,
                                tile_position=(0, 32 * hh),
                            )
                        for wp in range(NWP):
                            c0 = wp * 128
                            for hh in range(4):
                                h = 4 * hg + hh
                                esl = edm_sb[(hg, hh // 2)][:, hh % 2,
                                                            c0:c0 + 128]
                                nc.tensor.matmul(
                                    avps[32 * hh:32 * hh + 32, wp, :],
                                    v_sb[wp][:, h, :],
                                    esl,
                                    start=True, stop=True,
                                    tile_position=(0, 32 * hh),
                                )
                        recd = rcp.tile([128, NWP, 128], dt.float32, tag="recd")
                        nc.vector.reciprocal(recd, dnps)
                        attn = atp.tile([128, NWP, 128], dt.bfloat16, tag="attn")
                        nc.vector.tensor_mul(attn, avps, recd)
                        attn_sb.append(attn)

                    # ---- out projection + b_out, batched DMA per strip
                    fo4 = fop.tile([128, NWP, 256], dt.bfloat16, tag="fo4")
                    for wp in range(NWP):
                        ops = pvC.tile([128, 256], dt.float32, tag="psC")
                        nc.tensor.matmul(ops, attn_sb[0][:, wp, :], wout_sb[0],
                                         start=True, stop=False)
                        nc.tensor.matmul(ops, attn_sb[1][:, wp, :], wout_sb[1],
                                         start=False, stop=False)
                        nc.tensor.matmul(ops, ones_sb, bout_sb,
                                         start=False, stop=True)
                        nc.scalar.copy(fo4[:, wp, :], ops)
                    nc.sync.dma_start(
                        out=out[img, t0:t0 + STRIP, :].rearrange(
                            "(w p) c -> p w c", w=NWP),
                        in_=fo4)
    nc.compile()
    return nc


def _host_prep(x, w_qkv, w_out, b_out, pos_embedding):
    ws = WS
    scale = CH ** -0.5
    xs = x.reshape(B * L, H // ws, ws, W // ws, ws, C)
    xs = xs.transpose(0, 1, 3, 2, 4, 5).reshape(IMG, T_IMG, C)
    xT = np.ascontiguousarray(xs.transpose(0, 2, 1)).astype(BF16)

    wq = (w_qkv[:, 0:256] * scale).astype(BF16)
    wk = w_qkv[:, 256:512].astype(BF16)
    wqk = np.concatenate([wq, wk], axis=1)
    wv = w_qkv[:, 512:768].astype(BF16)

    ri = _relative_indices(ws)
    bias = pos_embedding[ri[:, :, 0], ri[:, :, 1]]  # [i, j]
    ebT = np.exp(bias.astype(np.float64)).T.astype(np.float32)  # [j, i]
    ebsuper = np.zeros((128, 128), np.float32)
    ebsuper[0:64, 0:64] = ebT
    ebsuper[64:128, 64:128] = ebT
    ebrep = np.tile(ebsuper, (1, 8)).astype(BF16)  # [128, 1024]

    ones32 = np.ones((128, 32), dtype=BF16)
    ones1 = np.ones((1, 128), dtype=BF16)
    bout = b_out.reshape(1, C).astype(BF16)

    return {
        "xT": xT,
        "wqk": np.ascontiguousarray(wqk),
        "wv": np.ascontiguousarray(wv),
        "wout": w_out.astype(BF16),
        "bout": bout,
        "ebrep": ebrep,
        "ones32": ones32,
        "ones1": ones1,
    }


def _make_in_maps(prep):
    in_maps = []
    for core in range(N_CORES):
        m = dict(prep)
        m["xT"] = np.ascontiguousarray(
            prep["xT"][core * IMG_PER_CORE:(core + 1) * IMG_PER_CORE])
        in_maps.append(m)
    return in_maps


def kernel(x, w_qkv, w_out, b_out, pos_embedding, window_size, **extra):
    from concourse.bass_utils import run_bass_kernel_spmd

    x = np.asarray(x, dtype=np.float32)
    w_qkv = np.asarray(w_qkv, dtype=np.float32)
    w_out = np.asarray(w_out, dtype=np.float32)
    b_out = np.asarray(b_out, dtype=np.float32)
    pos_embedding = np.asarray(pos_embedding, dtype=np.float32)

    prep = _host_prep(x, w_qkv, w_out, b_out, pos_embedding)

    if "nc" not in _CACHE:
        _CACHE["nc"] = _build_kernel()
    nc = _CACHE["nc"]

    in_maps = _make_in_maps(prep)

    res = run_bass_kernel_spmd(nc, in_maps, core_ids=list(range(N_CORES)))
    outs = [res.results[c]["out"] for c in range(N_CORES)]
    o = np.concatenate(outs, axis=0)  # [16, 4096, 256]
    o = o.reshape(B * L, H // WS, W // WS, WS, WS, C)
    o = o.transpose(0, 1, 3, 2, 4, 5).reshape(B, L, H, W, C)
    return np.ascontiguousarray(o.astype(np.float32))


# revision 11
# speedup vs baseline: 2.4139x; 2.4139x over previous
"""Trainium2 Bass kernel for BaseWindowAttention.

Problem (hardcoded): x [2,8,64,64,256] f32, w_qkv [256,768], w_out [256,256],
b_out [256], pos_embedding [15,15], window_size 8, heads 8, dim_head 32.

Strategy:
- Data parallel: 16 (b,l) images over 8 cores -> 2 images/core.
- Host: window-major channel-first bf16 transpose of x; fold softmax scale
  into w_q; precompute exp(bias) 2-window super-tile (off-diagonal zeros kill
  cross-window attention terms).
- Device per core, per strip of 512 tokens (8 windows):
  q/k projection ([o,t] layout), v projection ([t,o] layout, head-strided with
  an appended ones column for the softmax denominator), window-pair dots as
  4 row-group-packed [32,128]x[32,128] matmuls, ACT exp, DVE multiply by
  exp(bias) mask tile, AV matmul (fused denominator), reciprocal + broadcast
  normalize, PE transpose to [hc,t], out-projection with b_out folded in as a
  K=1 rank-1 matmul.
"""

import os
import sys
import numpy as np

sys.path.insert(0, "/opt/trn_rl_repo")
os.environ.setdefault("JAX_PLATFORMS", "")

import ml_dtypes

BF16 = ml_dtypes.bfloat16

B, L, H, W, C = 2, 8, 64, 64, 256
WS = 8
NHEADS = 8
CH = 32
N_CORES = 8
IMG = B * L                 # 16 images
IMG_PER_CORE = IMG // N_CORES
T_IMG = H * W               # 4096 tokens per image
STRIP = 512                 # tokens per strip (8 windows)
N_STRIPS = T_IMG // STRIP   # 8
NWP = STRIP // 128          # 4 window pairs per strip

_CACHE = {}


def _relative_indices(ws):
    idx = np.array([[i, j] for i in range(ws) for j in range(ws)])
    rel = idx[None, :, :] - idx[:, None, :] + ws - 1
    return rel


def _build_kernel(repeat=1):
    import concourse.bass as bass
    import concourse.mybir as mybir
    import concourse.tile as tile
    from concourse import bacc

    dt = mybir.dt
    nc = bacc.Bacc("TRN2", target_bir_lowering=False, debug=False)

    xT = nc.dram_tensor("xT", [IMG_PER_CORE, C, T_IMG], dt.bfloat16,
                        kind="ExternalInput").ap()
    wqk = nc.dram_tensor("wqk", [C, 512], dt.bfloat16, kind="ExternalInput").ap()
    wv = nc.dram_tensor("wv", [C, C], dt.bfloat16, kind="ExternalInput").ap()
    wout = nc.dram_tensor("wout", [C, C], dt.bfloat16, kind="ExternalInput").ap()
    bout = nc.dram_tensor("bout", [1, C], dt.bfloat16, kind="ExternalInput").ap()
    ebrep = nc.dram_tensor("ebrep", [128, 2048], dt.bfloat16,
                           kind="ExternalInput").ap()
    ident = nc.dram_tensor("ident", [128, 128], dt.bfloat16,
                           kind="ExternalInput").ap()
    ones1 = nc.dram_tensor("ones1", [1, 128], dt.bfloat16,
                           kind="ExternalInput").ap()
    out = nc.dram_tensor("out", [IMG_PER_CORE, T_IMG, C], dt.bfloat16,
                         kind="ExternalOutput").ap()

    EXP = mybir.ActivationFunctionType.Exp

    with tile.TileContext(nc) as tc:
        from contextlib import ExitStack
        with ExitStack() as ctx:
            consts = ctx.enter_context(tc.tile_pool(name="consts", bufs=1))
            xp = ctx.enter_context(tc.tile_pool(name="xp", bufs=3))
            qkp = ctx.enter_context(tc.tile_pool(name="qkp", bufs=8))
            vp = ctx.enter_context(tc.tile_pool(name="vp", bufs=8))
            ep = ctx.enter_context(tc.tile_pool(name="ep", bufs=3))
            anp = ctx.enter_context(tc.tile_pool(name="anp", bufs=3))
            aotp = ctx.enter_context(tc.tile_pool(name="aotp", bufs=4))
            rdp = ctx.enter_context(tc.tile_pool(name="rdp", bufs=3))
            fop = ctx.enter_context(tc.tile_pool(name="fop", bufs=3))
            psp = ctx.enter_context(tc.tile_pool(name="psp", bufs=1, space="PSUM"))

            # ---- constants into SBUF
            wqk_sb = []
            wv_sb = []
            wout_sb = []
            for kk in range(2):
                wqk_t = consts.tile([128, 512], dt.bfloat16, tag=f"wqk{kk}")
                nc.sync.dma_start(out=wqk_t, in_=wqk[kk * 128:(kk + 1) * 128, :])
                wqk_sb.append(wqk_t)
                wv_t = consts.tile([128, 256], dt.bfloat16, tag=f"wv{kk}")
                nc.sync.dma_start(out=wv_t, in_=wv[kk * 128:(kk + 1) * 128, :])
                wv_sb.append(wv_t)
                wout_t = consts.tile([128, 256], dt.bfloat16, tag=f"wout{kk}")
                nc.sync.dma_start(out=wout_t, in_=wout[kk * 128:(kk + 1) * 128, :])
                wout_sb.append(wout_t)
            eb_sb = consts.tile([128, 4, 512], dt.bfloat16, tag="eb")
            nc.sync.dma_start(out=eb_sb, in_=ebrep.rearrange("p (r c) -> p r c", r=4))
            id_sb = consts.tile([128, 128], dt.bfloat16, tag="id")
            nc.sync.dma_start(out=id_sb, in_=ident)
            ones_sb = consts.tile([1, 128], dt.bfloat16, tag="ones")
            nc.sync.dma_start(out=ones_sb, in_=ones1)
            bout_sb = consts.tile([1, 256], dt.bfloat16, tag="bout")
            nc.sync.dma_start(out=bout_sb, in_=bout)

            for img_rep in range(IMG_PER_CORE * repeat):
                img = img_rep % IMG_PER_CORE
                for s in range(N_STRIPS):
                    t0 = s * STRIP
                    xa = xp.tile([128, STRIP], dt.bfloat16, tag="xa")
                    nc.sync.dma_start(out=xa, in_=xT[img, 0:128, t0:t0 + STRIP])
                    xb = xp.tile([128, STRIP], dt.bfloat16, tag="xb")
                    nc.sync.dma_start(out=xb, in_=xT[img, 128:256, t0:t0 + STRIP])

                    # ---- q/k projection: out [o=128 (4 heads), t=512]
                    qk_sb = []
                    for ot in range(4):  # q(h0-3), q(h4-7), k(h0-3), k(h4-7)
                        qkps = psp.tile([128, STRIP], dt.float32, tag="qkps",
                                        bufs=int(os.environ.get("QKPS_BUFS", "1")))
                        nc.tensor.matmul(qkps, wqk_sb[0][:, ot * 128:(ot + 1) * 128],
                                         xa, start=True, stop=False)
                        nc.tensor.matmul(qkps, wqk_sb[1][:, ot * 128:(ot + 1) * 128],
                                         xb, start=False, stop=True)
                        qk_t = qkp.tile([128, STRIP], dt.bfloat16, tag="qk_t")
                        if ot % 2 == 0:
                            nc.vector.tensor_copy(qk_t, qkps)
                        else:
                            nc.scalar.copy(qk_t, qkps)
                        qk_sb.append(qk_t)

                    # ---- v projection: out [t=128, 8, 32] + ones col -> [128,8,33]
                    v_sb = []
                    for tb in range(NWP):
                        vps = psp.tile([128, NHEADS, CH], dt.float32,
                                       tag="smallps", bufs=int(os.environ.get("SMALLPS_BUFS", "3")))
                        nc.tensor.matmul(vps, xa[:, tb * 128:(tb + 1) * 128],
                                         wv_sb[0], start=True, stop=False)
                        nc.tensor.matmul(vps, xb[:, tb * 128:(tb + 1) * 128],
                                         wv_sb[1], start=False, stop=True)
                        v3 = vp.tile([128, NHEADS, CH + 1], dt.bfloat16, tag="v3")
                        if os.environ.get("MEMSET_ENG", "gp") == "gp":
                            nc.gpsimd.memset(v3[:, :, CH:CH + 1], 1.0)
                        else:
                            nc.vector.memset(v3[:, :, CH:CH + 1], 1.0)
                        nc.scalar.copy(v3[:, :, 0:CH], vps)
                        v_sb.append(v3)

                    # ---- attention: strip-batched dots -> exp -> mask-mult
                    # row-groups split across 2-bank half tiles: concurrent
                    # row tiles must not share a PSUM bank
                    edm_sb = {}
                    _eng = os.environ.get("EDM_ENG", "gp")
                    for hg in range(2):
                        for half in range(2):
                            dps = psp.tile([128, 2, 512], dt.float32,
                                           tag="dps", bufs=2)
                            for wp in range(NWP):
                                c0 = wp * 128
                                for r2 in range(2):
                                    rg = 2 * half + r2
                                    nc.tensor.matmul(
                                        dps[:, r2, c0:c0 + 128],
                                        qk_sb[2 + hg][32 * rg:32 * rg + 32,
                                                      c0:c0 + 128],
                                        qk_sb[hg][32 * rg:32 * rg + 32,
                                                  c0:c0 + 128],
                                        start=True, stop=True,
                                        tile_position=(32 * rg, 0),
                                    )
                            ed = ep.tile([128, 2, 512], dt.bfloat16, tag="ed", bufs=4)
                            nc.scalar.activation(ed, dps, EXP)
                            edm = ep.tile([128, 2, 512], dt.bfloat16, tag="edm", bufs=10)
                            if _eng == "dve" or (_eng == "mix" and hg == 1 and half == 1):
                                nc.vector.tensor_mul(edm, ed,
                                                     eb_sb[:, 2 * half:2 * half + 2, :])
                            else:
                                nc.gpsimd.tensor_mul(edm, ed,
                                                     eb_sb[:, 2 * half:2 * half + 2, :])
                            edm_sb[(hg, half)] = edm

                    for wp in range(NWP):
                        c0 = wp * 128
                        avps = psp.tile([128, NHEADS, CH + 1], dt.float32,
                                        tag="smallps", bufs=int(os.environ.get("SMALLPS_BUFS", "3")))
                        for hg in range(2):
                            for rg in range(4):
                                h = 4 * hg + rg
                                nc.tensor.matmul(
                                    avps[:, h, :],
                                    edm_sb[(hg, rg // 2)][:, rg % 2, c0:c0 + 128],
                                    v_sb[wp][:, h, :],
                                    start=True, stop=True,
                                )
                        # normalize: attnout = av * (1/den)
                        rd = rdp.tile([128, NHEADS, 1], dt.float32, tag="rd")
                        nc.vector.reciprocal(rd, avps[:, :, CH:CH + 1])
                        attn = anp.tile([128, NHEADS, CH], dt.bfloat16,
                                        tag="attn")
                        nc.vector.tensor_mul(attn, avps[:, :, 0:CH],
                                             rd.to_broadcast((128, NHEADS, CH)))

                        # transpose [t,hc] -> [hc,t]
                        tps = psp.tile([128, 2, 128], dt.bfloat16,
                                       tag="smallps", bufs=int(os.environ.get("SMALLPS_BUFS", "3")))
                        for half in range(2):
                            nc.tensor.transpose(
                                tps[:, half, :],
                                attn[:, half * 4:(half + 1) * 4, :], id_sb)
                        aot = aotp.tile([128, 2, 128], dt.bfloat16, tag="aot")
                        if os.environ.get("AOT_ENG", "dve") == "act":
                            nc.scalar.copy(aot, tps)
                        else:
                            nc.vector.tensor_copy(aot, tps)
                        aot_sb = [aot[:, 0, :], aot[:, 1, :]]

                        # out projection + b_out
                        ops = psp.tile([128, 256], dt.float32, tag="smallps", bufs=int(os.environ.get("SMALLPS_BUFS", "3")))
                        if os.environ.get("ABL_NOBIAS"):
                            nc.tensor.matmul(ops, aot_sb[0], wout_sb[0],
                                             start=True, stop=False)
                            nc.tensor.matmul(ops, aot_sb[1], wout_sb[1],
                                             start=False, stop=True)
                        else:
                            nc.tensor.matmul(ops, aot_sb[0], wout_sb[0],
                                             start=True, stop=False)
                            nc.tensor.matmul(ops, aot_sb[1], wout_sb[1],
                                             start=False, stop=False)
                            nc.tensor.matmul(ops, ones_sb, bout_sb,
                                             start=False, stop=True)
                        fo = fop.tile([128, 256], dt.bfloat16, tag="fo")
                        if os.environ.get("FO_ENG", "dve") == "dve":
                            nc.vector.tensor_copy(fo, ops)
                        else:
                            nc.scalar.copy(fo, ops)
                        nc.sync.dma_start(
                            out=out[img, t0 + c0:t0 + c0 + 128, :], in_=fo)
    nc.compile()
    return nc


def _host_prep(x, w_qkv, w_out, b_out, pos_embedding):
    ws = WS
    scale = CH ** -0.5
    xs = x.reshape(B * L, H // ws, ws, W // ws, ws, C)
    xs = xs.transpose(0, 1, 3, 2, 4, 5).reshape(IMG, T_IMG, C)
    xT = np.ascontiguousarray(xs.transpose(0, 2, 1)).astype(BF16)

    wq = (w_qkv[:, 0:256] * scale).astype(BF16)
    wk = w_qkv[:, 256:512].astype(BF16)
    wqk = np.concatenate([wq, wk], axis=1)
    wv = w_qkv[:, 512:768].astype(BF16)

    ri = _relative_indices(ws)
    bias = pos_embedding[ri[:, :, 0], ri[:, :, 1]]  # [i, j]
    ebT = np.exp(bias.astype(np.float64)).T.astype(np.float32)  # [j, i]
    ebsuper = np.zeros((128, 128), np.float32)
    ebsuper[0:64, 0:64] = ebT
    ebsuper[64:128, 64:128] = ebT
    ebrep = np.tile(ebsuper, (1, 16)).astype(BF16)

    ident = np.eye(128, dtype=BF16)
    ones1 = np.ones((1, 128), dtype=BF16)
    bout = b_out.reshape(1, C).astype(BF16)

    return {
        "xT": xT,
        "wqk": np.ascontiguousarray(wqk),
        "wv": np.ascontiguousarray(wv),
        "wout": w_out.astype(BF16),
        "bout": bout,
        "ebrep": ebrep,
        "ident": ident,
        "ones1": ones1,
    }


def _make_in_maps(prep):
    in_maps = []
    for core in range(N_CORES):
        m = dict(prep)
        m["xT"] = np.ascontiguousarray(
            prep["xT"][core * IMG_PER_CORE:(core + 1) * IMG_PER_CORE])
        in_maps.append(m)
    return in_maps


def kernel(x, w_qkv, w_out, b_out, pos_embedding, window_size, **extra):
    from concourse.bass_utils import run_bass_kernel_spmd

    x = np.asarray(x, dtype=np.float32)
    w_qkv = np.asarray(w_qkv, dtype=np.float32)
    w_out = np.asarray(w_out, dtype=np.float32)
    b_out = np.asarray(b_out, dtype=np.float32)
    pos_embedding = np.asarray(pos_embedding, dtype=np.float32)

    prep = _host_prep(x, w_qkv, w_out, b_out, pos_embedding)

    if "nc" not in _CACHE:
        _CACHE["nc"] = _build_kernel()
    nc = _CACHE["nc"]

    in_maps = _make_in_maps(prep)

    res = run_bass_kernel_spmd(nc, in_maps, core_ids=list(range(N_CORES)))
    outs = [res.results[c]["out"] for c in range(N_CORES)]
    o = np.concatenate(outs, axis=0)  # [16, 4096, 256]
    o = o.reshape(B * L, H // WS, W // WS, WS, WS, C)
    o = o.transpose(0, 1, 3, 2, 4, 5).reshape(B, L, H, W, C)
    return np.ascontiguousarray(o.astype(np.float32))



# revision 13
# speedup vs baseline: 3.0276x; 1.2542x over previous
"""Trainium2 Bass kernel for BaseWindowAttention.

Problem (hardcoded): x [2,8,64,64,256] f32, w_qkv [256,768], w_out [256,256],
b_out [256], pos_embedding [15,15], window_size 8, heads 8, dim_head 32.

Strategy (v2):
- Data parallel: 16 (b,l) images over 8 cores -> 2 images/core.
- Host: window-major channel-first bf16 transpose of x; fold softmax scale
  into w_q; precompute exp(bias) 2-window super-tile (off-diagonal zeros kill
  cross-window attention terms).
- Device per core, per strip of 512 tokens (8 windows):
  * q/k projection -> [o,t] layout; v projection -> [t,h,c] layout.
  * window-pair dots as row-group-packed [32,x]x[32,x] matmuls (keys on
    partitions), ACT exp, Pool multiply by exp(bias) mask tile.
  * AV computed TRANSPOSED: avT[hc,t] = v.T @ edm via v-stationary matmuls
    col-packed 4 heads per [128,4,128] PSUM tile (tile_position=(0,32h)).
    No PE transpose needed before the out-projection.
  * softmax denominators via all-ones [128,32] stationary matmuls into a
    second [128,4,128] PSUM tile (each head's den replicated across its 32
    rows -> partition-aligned normalize).
  * normalize: DVE reciprocal + DVE multiply -> attn[hc,t] bf16 in SBUF.
  * out-projection with b_out folded in as a K=1 rank-1 matmul; one batched
    output DMA per strip.
"""

import os
import sys
import numpy as np

sys.path.insert(0, "/opt/trn_rl_repo")
os.environ.setdefault("JAX_PLATFORMS", "")

import ml_dtypes

BF16 = ml_dtypes.bfloat16

B, L, H, W, C = 2, 8, 64, 64, 256
WS = 8
NHEADS = 8
CH = 32
N_CORES = 8
IMG = B * L                 # 16 images
IMG_PER_CORE = IMG // N_CORES
T_IMG = H * W               # 4096 tokens per image
STRIP = 512                 # tokens per strip (8 windows)
N_STRIPS = T_IMG // STRIP   # 8
NWP = STRIP // 128          # 4 window pairs per strip

_CACHE = {}


def _relative_indices(ws):
    idx = np.array([[i, j] for i in range(ws) for j in range(ws)])
    rel = idx[None, :, :] - idx[:, None, :] + ws - 1
    return rel


def _build_kernel(repeat=1):
    import concourse.bass as bass
    import concourse.mybir as mybir
    import concourse.tile as tile
    from concourse import bacc

    dt = mybir.dt
    nc = bacc.Bacc("TRN2", target_bir_lowering=False, debug=False)

    xT = nc.dram_tensor("xT", [IMG_PER_CORE, C, T_IMG], dt.bfloat16,
                        kind="ExternalInput").ap()
    wqk = nc.dram_tensor("wqk", [C, 512], dt.bfloat16, kind="ExternalInput").ap()
    wv = nc.dram_tensor("wv", [C, C], dt.bfloat16, kind="ExternalInput").ap()
    wout = nc.dram_tensor("wout", [C, C], dt.bfloat16, kind="ExternalInput").ap()
    bout = nc.dram_tensor("bout", [1, C], dt.bfloat16, kind="ExternalInput").ap()
    ebrep = nc.dram_tensor("ebrep", [128, 1024], dt.bfloat16,
                           kind="ExternalInput").ap()
    ones32 = nc.dram_tensor("ones32", [128, 32], dt.bfloat16,
                            kind="ExternalInput").ap()
    ones1 = nc.dram_tensor("ones1", [1, 128], dt.bfloat16,
                           kind="ExternalInput").ap()
    out = nc.dram_tensor("out", [IMG_PER_CORE, T_IMG, C], dt.bfloat16,
                         kind="ExternalOutput").ap()

    EXP = mybir.ActivationFunctionType.Exp

    def env(k, d):
        return int(os.environ.get(k, str(d)))

    with tile.TileContext(nc) as tc:
        from contextlib import ExitStack
        with ExitStack() as ctx:
            consts = ctx.enter_context(tc.tile_pool(name="consts", bufs=1))
            xp = ctx.enter_context(tc.tile_pool(name="xp", bufs=env("XP_BUFS", 3)))
            qkp = ctx.enter_context(tc.tile_pool(name="qkp", bufs=env("QKP_BUFS", 8)))
            vp = ctx.enter_context(tc.tile_pool(name="vp", bufs=env("VP_BUFS", 6)))
            edp = ctx.enter_context(tc.tile_pool(name="edp", bufs=env("EDP_BUFS", 4)))
            emp = ctx.enter_context(tc.tile_pool(name="emp", bufs=env("EMP_BUFS", 4)))
            rcp = ctx.enter_context(tc.tile_pool(name="rcp", bufs=env("RCP_BUFS", 3)))
            atp = ctx.enter_context(tc.tile_pool(name="atp", bufs=env("ATP_BUFS", 4)))
            fop = ctx.enter_context(tc.tile_pool(name="fop", bufs=env("FOP_BUFS", 3)))
            # PSUM: pvA {qk, v} 2 banks + pvB {dots} 4 banks + pvC {avT, den, ops} 2
            pvA = ctx.enter_context(tc.tile_pool(
                name="pvA", bufs=env("PSA_BUFS", 2), space="PSUM"))
            pvB = ctx.enter_context(tc.tile_pool(
                name="pvB", bufs=env("PSB_BUFS", 2), space="PSUM"))
            pvC = ctx.enter_context(tc.tile_pool(
                name="pvC", bufs=env("PSC_BUFS", 2), space="PSUM"))

            # ---- constants into SBUF
            wqk_sb, wv_sb, wout_sb = [], [], []
            for kk in range(2):
                wqk_t = consts.tile([128, 512], dt.bfloat16, tag=f"wqk{kk}")
                nc.sync.dma_start(out=wqk_t, in_=wqk[kk * 128:(kk + 1) * 128, :])
                wqk_sb.append(wqk_t)
                wv_t = consts.tile([128, 256], dt.bfloat16, tag=f"wv{kk}")
                nc.sync.dma_start(out=wv_t, in_=wv[kk * 128:(kk + 1) * 128, :])
                wv_sb.append(wv_t)
                wout_t = consts.tile([128, 256], dt.bfloat16, tag=f"wout{kk}")
                nc.sync.dma_start(out=wout_t, in_=wout[kk * 128:(kk + 1) * 128, :])
                wout_sb.append(wout_t)
            eb_sb = consts.tile([128, 2, 512], dt.bfloat16, tag="eb")
            nc.sync.dma_start(out=eb_sb, in_=ebrep.rearrange("p (r c) -> p r c", r=2))
            o32_sb = consts.tile([128, 32], dt.bfloat16, tag="o32")
            nc.sync.dma_start(out=o32_sb, in_=ones32)
            ones_sb = consts.tile([1, 128], dt.bfloat16, tag="ones")
            nc.sync.dma_start(out=ones_sb, in_=ones1)
            bout_sb = consts.tile([1, 256], dt.bfloat16, tag="bout")
            nc.sync.dma_start(out=bout_sb, in_=bout)

            for img_rep in range(IMG_PER_CORE * repeat):
                img = img_rep % IMG_PER_CORE
                for s in range(N_STRIPS):
                    t0 = s * STRIP
                    xa = xp.tile([128, STRIP], dt.bfloat16, tag="xa")
                    nc.sync.dma_start(out=xa, in_=xT[img, 0:128, t0:t0 + STRIP])
                    xb = xp.tile([128, STRIP], dt.bfloat16, tag="xb")
                    nc.sync.dma_start(out=xb, in_=xT[img, 128:256, t0:t0 + STRIP])

                    # ---- q/k projection: out [o=128 (4 heads), t=512]
                    # bf16 PSUM: halves DVE copy cost (2x mode on 16-bit src).
                    qk_sb = []
                    for ot in range(4):  # q(h0-3), q(h4-7), k(h0-3), k(h4-7)
                        qkps = pvA.tile([128, STRIP], dt.float32, tag="psA")
                        nc.tensor.matmul(qkps, wqk_sb[0][:, ot * 128:(ot + 1) * 128],
                                         xa, start=True, stop=False)
                        nc.tensor.matmul(qkps, wqk_sb[1][:, ot * 128:(ot + 1) * 128],
                                         xb, start=False, stop=True)
                        qk_t = qkp.tile([128, STRIP], dt.bfloat16, tag="qk_t")
                        nc.vector.tensor_copy(qk_t, qkps)
                        qk_sb.append(qk_t)

                    # ---- v projection: out [t=128, 8, 32]
                    v_sb = []
                    for tb in range(NWP):
                        vps = pvA.tile([128, NHEADS, CH], dt.float32, tag="psA")
                        nc.tensor.matmul(vps, xa[:, tb * 128:(tb + 1) * 128],
                                         wv_sb[0], start=True, stop=False)
                        nc.tensor.matmul(vps, xb[:, tb * 128:(tb + 1) * 128],
                                         wv_sb[1], start=False, stop=True)
                        v3 = vp.tile([128, NHEADS, CH], dt.bfloat16, tag="v3")
                        nc.vector.tensor_copy(v3, vps)
                        v_sb.append(v3)

                    # ---- dots -> exp -> mask-mult, per (hg, half)
                    # edm[j, i] layout: keys j on partitions.
                    edm_sb = {}
                    for hg in range(2):
                        for half in range(2):
                            dps = pvB.tile([128, 2, 512], dt.float32, tag="dps")
                            for wp in range(NWP):
                                c0 = wp * 128
                                for r2 in range(2):
                                    rg = 2 * half + r2
                                    nc.tensor.matmul(
                                        dps[:, r2, c0:c0 + 128],
                                        qk_sb[2 + hg][32 * rg:32 * rg + 32,
                                                      c0:c0 + 128],
                                        qk_sb[hg][32 * rg:32 * rg + 32,
                                                  c0:c0 + 128],
                                        start=True, stop=True,
                                        tile_position=(32 * rg, 0),
                                    )
                            ed = edp.tile([128, 2, 512], dt.bfloat16, tag="ed")
                            nc.scalar.activation(ed, dps, EXP)
                            edm = emp.tile([128, 2, 512], dt.bfloat16, tag="edm")
                            nc.gpsimd.tensor_mul(edm, ed,
                                                 eb_sb)
                            edm_sb[(hg, half)] = edm

                    # ---- avT + den: [hc, t] via v-stationary col-packed MMs
                    attn_sb = []
                    for hg in range(2):
                        avps = pvC.tile([128, NWP, 128], dt.float32, tag="psC")
                        dnps = pvC.tile([128, NWP, 128], dt.float32, tag="psC")
                        for hh in range(4):
                            # one N=512 den matmul covers all 4 window pairs
                            nc.tensor.matmul(
                                dnps[32 * hh:32 * hh + 32, :, :],
                                o32_sb,
                                edm_sb[(hg, hh // 2)][:, hh % 2, :],
                                start=True, stop=True,
                                tile_position=(0, 32 * hh),
                            )
                        for wp in range(NWP):
                            c0 = wp * 128
                            for hh in range(4):
                                h = 4 * hg + hh
                                esl = edm_sb[(hg, hh // 2)][:, hh % 2,
                                                            c0:c0 + 128]
                                nc.tensor.matmul(
                                    avps[32 * hh:32 * hh + 32, wp, :],
                                    v_sb[wp][:, h, :],
                                    esl,
                                    start=True, stop=True,
                                    tile_position=(0, 32 * hh),
                                )
                        recd = rcp.tile([128, NWP, 128], dt.float32, tag="recd")
                        nc.vector.reciprocal(recd, dnps)
                        attn = atp.tile([128, NWP, 128], dt.bfloat16, tag="attn")
                        nc.vector.tensor_mul(attn, avps, recd)
                        attn_sb.append(attn)

                    # ---- out projection + b_out, per-wp DMA
                    for wp in range(NWP):
                        c0 = wp * 128
                        ops = pvC.tile([128, 256], dt.float32, tag="psC")
                        nc.tensor.matmul(ops, attn_sb[0][:, wp, :], wout_sb[0],
                                         start=True, stop=False)
                        nc.tensor.matmul(ops, attn_sb[1][:, wp, :], wout_sb[1],
                                         start=False, stop=False)
                        nc.tensor.matmul(ops, ones_sb, bout_sb,
                                         start=False, stop=True)
                        fo = fop.tile([128, 256], dt.bfloat16, tag="fo")
                        if os.environ.get("FO_ENG", "act") == "dve":
                            nc.vector.tensor_copy(fo, ops)
                        else:
                            nc.scalar.copy(fo, ops)
                        nc.sync.dma_start(
                            out=out[img, t0 + c0:t0 + c0 + 128, :], in_=fo)
    nc.compile()
    return nc


def _host_prep(x, w_qkv, w_out, b_out, pos_embedding):
    ws = WS
    scale = CH ** -0.5
    xs = x.reshape(B * L, H // ws, ws, W // ws, ws, C)
    xs = xs.transpose(0, 1, 3, 2, 4, 5).reshape(IMG, T_IMG, C)
    xT = np.ascontiguousarray(xs.transpose(0, 2, 1)).astype(BF16)

    wq = (w_qkv[:, 0:256] * scale).astype(BF16)
    wk = w_qkv[:, 256:512].astype(BF16)
    wqk = np.concatenate([wq, wk], axis=1)
    wv = w_qkv[:, 512:768].astype(BF16)

    ri = _relative_indices(ws)
    bias = pos_embedding[ri[:, :, 0], ri[:, :, 1]]  # [i, j]
    ebT = np.exp(bias.astype(np.float64)).T.astype(np.float32)  # [j, i]
    ebsuper = np.zeros((128, 128), np.float32)
    ebsuper[0:64, 0:64] = ebT
    ebsuper[64:128, 64:128] = ebT
    ebrep = np.tile(ebsuper, (1, 8)).astype(BF16)  # [128, 1024]

    ones32 = np.ones((128, 32), dtype=BF16)
    ones1 = np.ones((1, 128), dtype=BF16)
    bout = b_out.reshape(1, C).astype(BF16)

    return {
        "xT": xT,
        "wqk": np.ascontiguousarray(wqk),
        "wv": np.ascontiguousarray(wv),
        "wout": w_out.astype(BF16),
        "bout": bout,
        "ebrep": ebrep,
        "ones32": ones32,
        "ones1": ones1,
    }


def _make_in_maps(prep):
    in_maps = []
    for core in range(N_CORES):
        m = dict(prep)
        m["xT"] = np.ascontiguousarray(
            prep["xT"][core * IMG_PER_CORE:(core + 1) * IMG_PER_CORE])
        in_maps.append(m)
    return in_maps


def kernel(x, w_qkv, w_out, b_out, pos_embedding, window_size, **extra):
    from concourse.bass_utils import run_bass_kernel_spmd

    x = np.asarray(x, dtype=np.float32)
    w_qkv = np.asarray(w_qkv, dtype=np.float32)
    w_out = np.asarray(w_out, dtype=np.float32)
    b_out = np.asarray(b_out, dtype=np.float32)
    pos_embedding = np.asarray(pos_embedding, dtype=np.float32)

    prep = _host_prep(x, w_qkv, w_out, b_out, pos_embedding)

    if "nc" not in _CACHE:
        _CACHE["nc"] = _build_kernel()
    nc = _CACHE["nc"]

    in_maps = _make_in_maps(prep)

    res = run_bass_kernel_spmd(nc, in_maps, core_ids=list(range(N_CORES)))
    outs = [res.results[c]["out"] for c in range(N_CORES)]
    o = np.concatenate(outs, axis=0)  # [16, 4096, 256]
    o = o.reshape(B * L, H // WS, W // WS, WS, WS, C)
    o = o.transpose(0, 1, 3, 2, 4, 5).reshape(B, L, H, W, C)
    return np.ascontiguousarray(o.astype(np.float32))
